# revision 32
# baseline (speedup 1.0000x reference)
"""Trainium2 Bass kernel for nn_CustomCLIP_11407433138213 (moe_routing).

Math (per sample b with domain n = labels[b]):
    h   = relu(x @ W1[n])                 [R]
    a   = relu(h @ W2[n])                 [D]
    f   = 0.2*a + 0.8*x                   [D]
    out = exp(ls) * (f/||f||) @ T^T       [N_TXT]

Strategy: data-parallel over batch, 8 cores x 2048 rows, but the HOST
first sorts rows by domain label so each 512-row block is (nearly)
single-domain.  The device then runs exactly ONE expert per block with
per-block gathered weights -- no dense 3-expert compute, no masks.
Blocks that straddle a domain boundary (<= 2 in the whole batch, by
pigeonhole) are computed with the block's majority expert on device and
the few minority rows are recomputed exactly on the host and patched in
at unshard time.

Per row-block of 512 (all transposed: samples on the free dim):
    - mm1: hT = W1[b]^T XT   (2 M-tiles x 8 K-chunks), relu -> g (ACT)
    - mm2: pa = 0.25*W2[b]^T g  (8 M x 2 K); ft = relu(pa)+XT  (DVE)
    - norm: s = colsum(ft^2) via ones-matmul; iv = rsqrt(s*exp(-2ls));
      bcast(iv) via rank-1 matmul
    - mm3: logitsT = TT^T ft (11 M x 8 K); ob = logitsT * bcast(iv)
Everything flows through the PE in bf16 (1 cyc/row, FWL weight loads)
with fp32 PSUM accumulation; fp32 would-be-4x-slower, fp32r same speed
but 2x the DMA.  122 matmuls/block vs 186 for the dense-3-expert
baseline.  Output is written bf16 and upcast on the host.
"""

import contextlib
import sys

sys.path.insert(0, "/opt/trn_rl_repo")

import ml_dtypes
import numpy as np

import concourse.bass as bass  # noqa: F401  (registers engine types)
import concourse.mybir as mybir
import concourse.tile as tile
from concourse import bacc
from concourse.bass_utils import run_bass_kernel_spmd

# Problem constants (hardcoded per task contract).
B, D, R, ND, NT = 16384, 1024, 256, 3, 1380
NC = 8                    # cores
BPC = B // NC             # rows per core = 2048
RB = 512                  # row-block (matmul moving dim)
NB = BPC // RB            # row-blocks per core = 4
NBLK = B // RB            # 32 global blocks
KD = D // 128             # 8 contraction chunks over D
KR = R // 128             # 2 chunks over R
MR = R // 128             # 2 M-chunks over R
NTP = 1408                # text padded to 11*128
TTI = NTP // 128          # 11 text chunks

F32 = mybir.dt.float32
F32R = mybir.dt.float32r
# bf16 tiles produced garbage on this stack (engine-written 16-bit tiles
# corrupt); f32r runs the PE at the same 1 cycle/row for N>=256.
BF16 = mybir.dt.float32r
BF16NP = np.float32
import os


def build_program():
    nc = bacc.Bacc(
        "TRN2",
        target_bir_lowering=False,
        debug=False,
        enable_asserts=True,
        num_devices=NC,
    )
    # bf16 payloads are PACKED into f32-declared DRAM tensors (2 bf16 per
    # f32) and bitcast to bf16 on the device side: 2-byte-declared DRAM
    # params get corrupted by the input binding (even elements clobbered),
    # while the 4-byte path is the one the working f32r baseline used.
    xt = nc.declare_dram_parameter("xt", [D, BPC], F32R, isOutput=False)
    w1b = nc.declare_dram_parameter("w1b", [NB, D, R], F32R, isOutput=False)
    w2b = nc.declare_dram_parameter("w2b", [NB, R, D], F32R, isOutput=False)
    tt = nc.declare_dram_parameter("tt", [D, NTP], F32R, isOutput=False)
    sc = nc.declare_dram_parameter("sc", [1, 1], F32, isOutput=False)
    oc = nc.declare_dram_parameter("oc", [128, 1], F32R, isOutput=False)
    orow = nc.declare_dram_parameter("orow", [1, 128], F32R, isOutput=False)
    ot = nc.declare_dram_parameter("ot", [NTP, BPC], F32, isOutput=True)
    DBG = os.environ.get("KDBG") == "1"
    if DBG:
        dbg_g = nc.declare_dram_parameter("dbg_g", [128, RB], BF16, isOutput=True)
        dbg_ft = nc.declare_dram_parameter("dbg_ft", [128, RB], BF16, isOutput=True)
        dbg_w = nc.declare_dram_parameter("dbg_w", [128, R], BF16, isOutput=True)
        dbg_xb = nc.declare_dram_parameter("dbg_xb", [128, RB], BF16, isOutput=True)

    with tile.TileContext(nc) as tc, contextlib.ExitStack() as ctx:
        cst = ctx.enter_context(tc.tile_pool(name="cst", bufs=1))
        p_w1 = ctx.enter_context(tc.tile_pool(name="p_w1", bufs=2 * KD))
        p_w2 = ctx.enter_context(tc.tile_pool(name="p_w2", bufs=2 * KR))
        p_xb = ctx.enter_context(tc.tile_pool(name="p_xb", bufs=2 * KD))
        p_g = ctx.enter_context(tc.tile_pool(name="p_g", bufs=2 * MR))
        p_fp = ctx.enter_context(tc.tile_pool(name="p_fp", bufs=2 * KD))
        p_sq = ctx.enter_context(tc.tile_pool(name="p_sq", bufs=3))
        p_acc = ctx.enter_context(tc.tile_pool(name="p_acc", bufs=2))
        p_pbs = ctx.enter_context(tc.tile_pool(name="p_pbs", bufs=2))
        p_ob = ctx.enter_context(tc.tile_pool(name="p_ob", bufs=3))
        p_nrm = ctx.enter_context(tc.tile_pool(name="p_nrm", bufs=2))

        ps_h = ctx.enter_context(tc.tile_pool(name="ps_h", bufs=2, space="PSUM"))
        ps_a = ctx.enter_context(tc.tile_pool(name="ps_a", bufs=2, space="PSUM"))
        ps_s = ctx.enter_context(tc.tile_pool(name="ps_s", bufs=1, space="PSUM"))
        ps_l = ctx.enter_context(tc.tile_pool(name="ps_l", bufs=3, space="PSUM"))

        # ---- constant tiles -------------------------------------------
        ttt = [
            cst.tile([128, NTP], BF16, name=f"tt_{k}", tag=f"tt_{k}")
            for k in range(KD)
        ]
        ones_col = cst.tile([128, 1], F32R, name="ones_col", tag="ones_col")
        ones_row = cst.tile([1, 128], F32R, name="ones_row", tag="ones_row")
        sct = cst.tile([1, 1], F32, name="sct", tag="sct")

        # per-block live tiles
        S = [dict() for _ in range(NB)]

        def emit_consts():
            nc.sync.dma_start(ones_col[:], oc[:])
            nc.sync.dma_start(ones_row[:], orow[:])
            nc.sync.dma_start(sct[:], sc[:])

        def emit_tt(k):
            nc.sync.dma_start(ttt[k][:], tt[k * 128 : (k + 1) * 128, :])

        def emit_wx(b, with_tt=False):
            c0 = b * RB
            wt, xb = [], []
            for k in range(KD):
                w = p_w1.tile([128, R], BF16, name="w1", tag="w1")
                nc.sync.dma_start(w[:], w1b[b, k * 128 : (k + 1) * 128, :])
                wt.append(w)
                t = p_xb.tile([128, RB], BF16, name="xb", tag="xb")
                nc.sync.dma_start(t[:], xt[k * 128 : (k + 1) * 128, c0 : c0 + RB])
                xb.append(t)
                if with_tt:
                    emit_tt(k)
            w2t = []
            for r in range(KR):
                w = p_w2.tile([128, D], BF16, name="w2", tag="w2")
                nc.sync.dma_start(w[:], w2b[b, r * 128 : (r + 1) * 128, :])
                w2t.append(w)
            S[b]["wt"] = wt
            S[b]["xb"] = xb
            S[b]["w2t"] = w2t

        def emit_mm1_g(b):
            wt, xb = S[b]["wt"], S[b]["xb"]
            g = [None] * MR
            for m in range(MR):
                ph = ps_h.tile([128, RB], F32, name="ph", tag="ph")
                for k in range(KD):
                    nc.tensor.matmul(
                        ph[:],
                        wt[k][:, m * 128 : (m + 1) * 128],
                        xb[k][:],
                        start=(k == 0),
                        stop=(k == KD - 1),
                    )
                gt = p_g.tile([128, RB], BF16, name="g", tag="g")
                nc.scalar.activation(
                    gt[:], ph[:], mybir.ActivationFunctionType.Relu
                )
                g[m] = gt
            S[b]["g"] = g

        def emit_mm2(b):
            xb, g, w2t = S[b]["xb"], S[b]["g"], S[b]["w2t"]
            fp = []
            for d in range(KD):
                pa = ps_a.tile([128, RB], F32, name="pa", tag="pa")
                for r in range(KR):
                    nc.tensor.matmul(
                        pa[:],
                        w2t[r][:, d * 128 : (d + 1) * 128],
                        g[r][:],
                        start=(r == 0),
                        stop=(r == KR - 1),
                    )
                ft = p_fp.tile([128, RB], BF16, name="fp", tag="fp")
                nc.vector.scalar_tensor_tensor(
                    ft[:],
                    pa[:],
                    0.0,
                    xb[d][:].bitcast(F32),
                    mybir.AluOpType.max,
                    mybir.AluOpType.add,
                )
                fp.append(ft)
                st = p_sq.tile([128, RB], F32, name="sq", tag="sq")
                nc.scalar.square(st[:], ft[:].bitcast(F32))
                if d == 0:
                    acc = p_acc.tile([128, RB], F32, name="acc", tag="acc")
                    nc.gpsimd.tensor_copy(acc[:], st[:])
                elif d < KD - 1:
                    nc.gpsimd.tensor_add(acc[:], acc[:], st[:])
                else:
                    accm = p_acc.tile([128, RB], F32R, name="accm", tag="accm")
                    nc.gpsimd.tensor_add(accm[:], acc[:], st[:])
            S[b]["fp"] = fp
            S[b]["accm"] = accm

        def emit_ps_norm(b):
            accm = S[b]["accm"]
            ps = ps_s.tile([1, RB], F32, name="ps", tag="ps")
            nc.tensor.matmul(ps[:], ones_col[:], accm[:], start=True, stop=True)
            iv = p_nrm.tile([1, RB], F32R, name="iv", tag="iv")
            nc.scalar.activation(
                iv[:],
                ps[:],
                mybir.ActivationFunctionType.Abs_reciprocal_sqrt,
                scale=sct[:],
            )
            S[b]["iv"] = iv

        def emit_pb(b):
            iv = S[b]["iv"]
            pb = ps_l.tile([128, RB], F32, name="pl", tag="pl")
            nc.tensor.matmul(pb[:], ones_row[:], iv[:], start=True, stop=True)
            pbs = p_pbs.tile([128, RB], F32, name="pbs", tag="pbs")
            nc.scalar.copy(pbs[:], pb[:])
            S[b]["pbs"] = pbs

        def emit_mm3(b):
            # k-outer over 2-ttile panels: block 0's mm3 only needs ttt[k]
            # at its k-th step, so the big tt load streams behind it
            # instead of stalling mm3(0) until all of tt has landed.
            c0 = b * RB
            fp = S[b]["fp"]
            pbs = S[b]["pbs"]
            t_i = 0
            while t_i < TTI:
                n_p = min(2, TTI - t_i)
                pls = [
                    ps_l.tile([128, RB], F32, name="pl", tag="pl")
                    for _ in range(n_p)
                ]
                for k in range(KD):
                    for j in range(n_p):
                        nc.tensor.matmul(
                            pls[j][:],
                            ttt[k][:, (t_i + j) * 128 : (t_i + j + 1) * 128],
                            fp[k][:],
                            start=(k == 0),
                            stop=(k == KD - 1),
                        )
                for j in range(n_p):
                    ob = p_ob.tile([128, RB], F32, name="ob", tag="ob")
                    nc.vector.tensor_mul(ob[:], pls[j][:], pbs[:])
                    nc.sync.dma_start(
                        ot[(t_i + j) * 128 : (t_i + j + 1) * 128, c0 : c0 + RB],
                        ob[:],
                    )
                t_i += n_p
            S[b].clear()

        def emit_dbg(b):
            if not DBG or b != 0:
                return
            nc.sync.dma_start(dbg_g[:, :], S[b]["g"][0][:])
            nc.sync.dma_start(dbg_w[:, :], S[b]["wt"][0][:])
            nc.sync.dma_start(dbg_xb[:, :], S[b]["xb"][0][:])

        # ---- emission schedule (software pipelined) ---------------------
        # wx(0) first so mm1 starts ~1us in; the big tt tiles stream behind
        # it, covered by mm1+mm2 of block 0 before mm3(0) needs them.
        emit_consts()
        emit_wx(0)
        emit_wx(1)
        for k in range(KD):
            emit_tt(k)
        emit_mm1_g(0)
        emit_mm2(0)
        emit_ps_norm(0)
        for b in range(NB):
            emit_dbg(b)
            if b + 2 < NB:
                emit_wx(b + 2)
            if b + 1 < NB:
                emit_mm1_g(b + 1)
            emit_pb(b)
            if b + 1 < NB:
                emit_mm2(b + 1)
            emit_mm3(b)
            if b + 1 < NB:
                emit_ps_norm(b + 1)

    nc.compile()
    return nc


_NC_CACHE = None


def _get_program():
    global _NC_CACHE
    if _NC_CACHE is None:
        _NC_CACHE = build_program()
    return _NC_CACHE


def _prep(image_features, domain_labels, W1, W2, text_features, logit_scale):
    """Host-side: sort rows by domain, gather per-block weights, compute
    exact logits for the few rows that land in a mixed block with the
    wrong expert."""
    x = np.asarray(image_features, dtype=np.float32)
    labels = np.asarray(domain_labels).astype(np.int64)
    W1 = np.asarray(W1, dtype=np.float32)
    W2 = np.asarray(W2, dtype=np.float32)
    T = np.asarray(text_features, dtype=np.float32)
    ls = float(np.asarray(logit_scale))

    perm = np.argsort(labels, kind="stable")
    lab_s = labels[perm]
    blk_dom = np.empty(NBLK, dtype=np.int64)
    fix_sorted = np.zeros(B, dtype=bool)
    for i in range(NBLK):
        seg = lab_s[i * RB : (i + 1) * RB]
        counts = np.bincount(seg, minlength=ND)
        m = int(counts.argmax())
        blk_dom[i] = m
        if counts[m] != RB:
            fix_sorted[i * RB : (i + 1) * RB] = seg != m

    x_s = x[perm]                                           # [B, D]
    xt_full = np.ascontiguousarray(x_s.T).astype(BF16NP)    # [D, B]
    w1_b = W1.astype(BF16NP)                                # [ND, D, R]
    w2_b = np.ascontiguousarray(0.25 * W2).astype(BF16NP)   # [ND, R, D]
    ttp = np.zeros((D, NTP), dtype=np.float32)
    ttp[:, :NT] = T.T
    tt_b = ttp.astype(BF16NP)
    sc = np.array([[np.exp(-2.0 * ls)]], dtype=np.float32)
    oc = np.ones((128, 1), dtype=np.float32)
    orow = np.ones((1, 128), dtype=np.float32)

    in_maps = []
    for c in range(NC):
        cols = slice(c * BPC, (c + 1) * BPC)
        doms = blk_dom[c * NB : (c + 1) * NB]
        in_maps.append(
            {
                "xt": np.ascontiguousarray(xt_full[:, cols]),
                "w1b": np.ascontiguousarray(w1_b[doms]),
                "w2b": np.ascontiguousarray(w2_b[doms]),
                "tt": tt_b,
                "sc": sc,
                "oc": oc,
                "orow": orow,
            }
        )

    # exact recompute for minority rows of mixed blocks
    fix_orig = perm[fix_sorted]
    fixed = np.empty((fix_orig.size, NT), dtype=np.float32)
    if fix_orig.size:
        xe = x[fix_orig]
        le = labels[fix_orig]
        for dcur in range(ND):
            m = le == dcur
            if not m.any():
                continue
            xm = xe[m]
            h = np.maximum(xm @ W1[dcur], 0.0)
            a = np.maximum(h @ W2[dcur], 0.0)
            f = 0.2 * a + 0.8 * xm
            f /= np.linalg.norm(f, axis=1, keepdims=True)
            fixed[m] = np.exp(ls) * (f @ T.T)
    return in_maps, perm, fix_orig, fixed


def make_in_maps(image_features, domain_labels, W1, W2, text_features, logit_scale):
    in_maps, _, _, _ = _prep(
        image_features, domain_labels, W1, W2, text_features, logit_scale
    )
    return in_maps


def kernel(image_features, domain_labels, W1, W2, text_features, logit_scale, **kw):
    in_maps, perm, fix_orig, fixed = _prep(
        image_features, domain_labels, W1, W2, text_features, logit_scale
    )
    nc = _get_program()
    res = run_bass_kernel_spmd(nc, in_maps, list(range(NC)))

    out_sorted = np.empty((B, NT), dtype=np.float32)
    for c in range(NC):
        out_sorted[c * BPC : (c + 1) * BPC, :] = res.results[c]["ot"][:NT, :].T
    out = np.empty((B, NT), dtype=np.float32)
    out[perm] = out_sorted
    if fix_orig.size:
        out[fix_orig] = fixed
    return out


# revision 34
# speedup vs baseline: 1.0230x; 1.0230x over previous
"""Trainium2 Bass kernel for nn_CustomCLIP_11407433138213 (moe_routing).

Math (per sample b with domain n = labels[b]):
    h   = relu(x @ W1[n])                 [R]
    a   = relu(h @ W2[n])                 [D]
    f   = 0.2*a + 0.8*x                   [D]
    out = exp(ls) * (f/||f||) @ T^T       [N_TXT]

Strategy: data-parallel over batch, 8 cores x 2048 rows, but the HOST
first sorts rows by domain label so each 512-row block is (nearly)
single-domain.  The device then runs exactly ONE expert per block with
per-block gathered weights -- no dense 3-expert compute, no masks.
Blocks that straddle a domain boundary (<= 2 in the whole batch, by
pigeonhole) are computed with the block's majority expert on device and
the few minority rows are recomputed exactly on the host and patched in
at unshard time.

Per row-block of 512 (all transposed: samples on the free dim):
    - mm1: hT = W1[b]^T XT   (2 M-tiles x 8 K-chunks), relu -> g (ACT)
    - mm2: pa = 0.25*W2[b]^T g  (8 M x 2 K); ft = relu(pa)+XT  (DVE)
    - norm: s = colsum(ft^2) via ones-matmul; iv = rsqrt(s*exp(-2ls));
      bcast(iv) via rank-1 matmul
    - mm3: logitsT = TT^T ft (11 M x 8 K); ob = logitsT * bcast(iv)
Everything flows through the PE in bf16 (1 cyc/row, FWL weight loads)
with fp32 PSUM accumulation; fp32 would-be-4x-slower, fp32r same speed
but 2x the DMA.  122 matmuls/block vs 186 for the dense-3-expert
baseline.  Output is written bf16 and upcast on the host.
"""

import contextlib
import sys

sys.path.insert(0, "/opt/trn_rl_repo")

import ml_dtypes
import numpy as np

import concourse.bass as bass  # noqa: F401  (registers engine types)
import concourse.mybir as mybir
import concourse.tile as tile
from concourse import bacc
from concourse.bass_utils import run_bass_kernel_spmd

# Problem constants (hardcoded per task contract).
B, D, R, ND, NT = 16384, 1024, 256, 3, 1380
NC = 8                    # cores
BPC = B // NC             # rows per core = 2048
RB = 512                  # row-block (matmul moving dim)
NB = BPC // RB            # row-blocks per core = 4
NBLK = B // RB            # 32 global blocks
KD = D // 128             # 8 contraction chunks over D
KR = R // 128             # 2 chunks over R
MR = R // 128             # 2 M-chunks over R
NTP = 1408                # text padded to 11*128
TTI = NTP // 128          # 11 text chunks

F32 = mybir.dt.float32
F32R = mybir.dt.float32r
# bf16 tiles produced garbage on this stack (engine-written 16-bit tiles
# corrupt); f32r runs the PE at the same 1 cycle/row for N>=256.
BF16 = mybir.dt.float32r
BF16NP = np.float32
import os


def build_program():
    nc = bacc.Bacc(
        "TRN2",
        target_bir_lowering=False,
        debug=False,
        enable_asserts=True,
        num_devices=NC,
    )
    # bf16 payloads are PACKED into f32-declared DRAM tensors (2 bf16 per
    # f32) and bitcast to bf16 on the device side: 2-byte-declared DRAM
    # params get corrupted by the input binding (even elements clobbered),
    # while the 4-byte path is the one the working f32r baseline used.
    xt = nc.declare_dram_parameter("xt", [D, BPC], F32R, isOutput=False)
    w1b = nc.declare_dram_parameter("w1b", [NB, D, R], F32R, isOutput=False)
    w2b = nc.declare_dram_parameter("w2b", [NB, R, D], F32R, isOutput=False)
    tt = nc.declare_dram_parameter("tt", [D, NTP], F32R, isOutput=False)
    sc = nc.declare_dram_parameter("sc", [1, 1], F32, isOutput=False)
    oc = nc.declare_dram_parameter("oc", [128, 1], F32R, isOutput=False)
    orow = nc.declare_dram_parameter("orow", [1, 128], F32R, isOutput=False)
    ot = nc.declare_dram_parameter("ot", [NTP, BPC], F32, isOutput=True)
    DBG = os.environ.get("KDBG") == "1"
    if DBG:
        dbg_g = nc.declare_dram_parameter("dbg_g", [128, RB], BF16, isOutput=True)
        dbg_ft = nc.declare_dram_parameter("dbg_ft", [128, RB], BF16, isOutput=True)
        dbg_w = nc.declare_dram_parameter("dbg_w", [128, R], BF16, isOutput=True)
        dbg_xb = nc.declare_dram_parameter("dbg_xb", [128, RB], BF16, isOutput=True)

    with tile.TileContext(nc) as tc, contextlib.ExitStack() as ctx:
        cst = ctx.enter_context(tc.tile_pool(name="cst", bufs=1))
        p_w1 = ctx.enter_context(tc.tile_pool(name="p_w1", bufs=2 * KD))
        p_w2 = ctx.enter_context(tc.tile_pool(name="p_w2", bufs=2 * KR))
        p_xb = ctx.enter_context(tc.tile_pool(name="p_xb", bufs=2 * KD))
        p_g = ctx.enter_context(tc.tile_pool(name="p_g", bufs=2 * MR))
        p_fp = ctx.enter_context(tc.tile_pool(name="p_fp", bufs=2 * KD))
        p_sq = ctx.enter_context(tc.tile_pool(name="p_sq", bufs=3))
        p_acc = ctx.enter_context(tc.tile_pool(name="p_acc", bufs=2))
        p_pbs = ctx.enter_context(tc.tile_pool(name="p_pbs", bufs=2))
        p_ob = ctx.enter_context(tc.tile_pool(name="p_ob", bufs=3))
        p_nrm = ctx.enter_context(tc.tile_pool(name="p_nrm", bufs=2))

        ps_h = ctx.enter_context(tc.tile_pool(name="ps_h", bufs=2, space="PSUM"))
        ps_a = ctx.enter_context(tc.tile_pool(name="ps_a", bufs=2, space="PSUM"))
        ps_s = ctx.enter_context(tc.tile_pool(name="ps_s", bufs=1, space="PSUM"))
        ps_l = ctx.enter_context(tc.tile_pool(name="ps_l", bufs=3, space="PSUM"))

        # ---- constant tiles -------------------------------------------
        ttt = [
            cst.tile([128, NTP], BF16, name=f"tt_{k}", tag=f"tt_{k}")
            for k in range(KD)
        ]
        ones_col = cst.tile([128, 1], F32R, name="ones_col", tag="ones_col")
        ones_row = cst.tile([1, 128], F32R, name="ones_row", tag="ones_row")
        sct = cst.tile([1, 1], F32, name="sct", tag="sct")

        # per-block live tiles
        S = [dict() for _ in range(NB)]

        def emit_consts():
            nc.sync.dma_start(ones_col[:], oc[:])
            nc.sync.dma_start(ones_row[:], orow[:])
            nc.sync.dma_start(sct[:], sc[:])

        def emit_tt(k):
            nc.sync.dma_start(ttt[k][:], tt[k * 128 : (k + 1) * 128, :])

        def emit_wx(b, with_tt=False):
            c0 = b * RB
            wt, xb = [], []
            for k in range(KD):
                w = p_w1.tile([128, R], BF16, name="w1", tag="w1")
                nc.sync.dma_start(w[:], w1b[b, k * 128 : (k + 1) * 128, :])
                wt.append(w)
                t = p_xb.tile([128, RB], BF16, name="xb", tag="xb")
                nc.sync.dma_start(t[:], xt[k * 128 : (k + 1) * 128, c0 : c0 + RB])
                xb.append(t)
                if with_tt:
                    emit_tt(k)
            w2t = []
            for r in range(KR):
                w = p_w2.tile([128, D], BF16, name="w2", tag="w2")
                nc.sync.dma_start(w[:], w2b[b, r * 128 : (r + 1) * 128, :])
                w2t.append(w)
            S[b]["wt"] = wt
            S[b]["xb"] = xb
            S[b]["w2t"] = w2t

        def emit_mm1_g(b):
            wt, xb = S[b]["wt"], S[b]["xb"]
            g = [None] * MR
            for m in range(MR):
                ph = ps_h.tile([128, RB], F32, name="ph", tag="ph")
                for k in range(KD):
                    nc.tensor.matmul(
                        ph[:],
                        wt[k][:, m * 128 : (m + 1) * 128],
                        xb[k][:],
                        start=(k == 0),
                        stop=(k == KD - 1),
                    )
                gt = p_g.tile([128, RB], BF16, name="g", tag="g")
                nc.scalar.activation(
                    gt[:], ph[:], mybir.ActivationFunctionType.Relu
                )
                g[m] = gt
            S[b]["g"] = g

        def emit_mm2(b):
            xb, g, w2t = S[b]["xb"], S[b]["g"], S[b]["w2t"]
            fp = []
            for d in range(KD):
                pa = ps_a.tile([128, RB], F32, name="pa", tag="pa")
                for r in range(KR):
                    nc.tensor.matmul(
                        pa[:],
                        w2t[r][:, d * 128 : (d + 1) * 128],
                        g[r][:],
                        start=(r == 0),
                        stop=(r == KR - 1),
                    )
                ft = p_fp.tile([128, RB], BF16, name="fp", tag="fp")
                nc.vector.scalar_tensor_tensor(
                    ft[:],
                    pa[:],
                    0.0,
                    xb[d][:].bitcast(F32),
                    mybir.AluOpType.max,
                    mybir.AluOpType.add,
                )
                fp.append(ft)
                st = p_sq.tile([128, RB], F32, name="sq", tag="sq")
                nc.scalar.square(st[:], ft[:].bitcast(F32))
                if d == 0:
                    acc = p_acc.tile([128, RB], F32, name="acc", tag="acc")
                    nc.gpsimd.tensor_copy(acc[:], st[:])
                elif d < KD - 1:
                    nc.gpsimd.tensor_add(acc[:], acc[:], st[:])
                else:
                    accm = p_acc.tile([128, RB], F32R, name="accm", tag="accm")
                    nc.gpsimd.tensor_add(accm[:], acc[:], st[:])
            S[b]["fp"] = fp
            S[b]["accm"] = accm

        def emit_ps_norm(b):
            accm = S[b]["accm"]
            ps = ps_s.tile([1, RB], F32, name="ps", tag="ps")
            nc.tensor.matmul(ps[:], ones_col[:], accm[:], start=True, stop=True)
            iv = p_nrm.tile([1, RB], F32R, name="iv", tag="iv")
            nc.scalar.activation(
                iv[:],
                ps[:],
                mybir.ActivationFunctionType.Abs_reciprocal_sqrt,
                scale=sct[:],
            )
            S[b]["iv"] = iv

        def emit_pb(b):
            iv = S[b]["iv"]
            pb = ps_l.tile([128, RB], F32, name="pl", tag="pl")
            nc.tensor.matmul(pb[:], ones_row[:], iv[:], start=True, stop=True)
            pbs = p_pbs.tile([128, RB], F32, name="pbs", tag="pbs")
            nc.scalar.copy(pbs[:], pb[:])
            S[b]["pbs"] = pbs

        def emit_mm3(b):
            c0 = b * RB
            fp = S[b]["fp"]
            pbs = S[b]["pbs"]
            for t_i in range(TTI):
                pl = ps_l.tile([128, RB], F32, name="pl", tag="pl")
                for k in range(KD):
                    nc.tensor.matmul(
                        pl[:],
                        ttt[k][:, t_i * 128 : (t_i + 1) * 128],
                        fp[k][:],
                        start=(k == 0),
                        stop=(k == KD - 1),
                    )
                ob = p_ob.tile([128, RB], F32, name="ob", tag="ob")
                nc.vector.tensor_mul(ob[:], pl[:], pbs[:])
                nc.sync.dma_start(
                    ot[t_i * 128 : (t_i + 1) * 128, c0 : c0 + RB], ob[:]
                )
            S[b].clear()

        def emit_dbg(b):
            if not DBG or b != 0:
                return
            nc.sync.dma_start(dbg_g[:, :], S[b]["g"][0][:])
            nc.sync.dma_start(dbg_w[:, :], S[b]["wt"][0][:])
            nc.sync.dma_start(dbg_xb[:, :], S[b]["xb"][0][:])

        # ---- emission schedule (software pipelined) ---------------------
        # wx(0) first so mm1 starts ~1us in; the big tt tiles stream behind
        # it, covered by mm1+mm2 of block 0 before mm3(0) needs them.
        emit_consts()
        emit_wx(0)
        for k in range(KD):
            emit_tt(k)
        emit_wx(1)
        emit_mm1_g(0)
        emit_mm2(0)
        emit_ps_norm(0)
        for b in range(NB):
            emit_dbg(b)
            if b + 2 < NB:
                emit_wx(b + 2)
            if b + 1 < NB:
                emit_mm1_g(b + 1)
            emit_pb(b)
            if b + 1 < NB:
                emit_mm2(b + 1)
            emit_mm3(b)
            if b + 1 < NB:
                emit_ps_norm(b + 1)

    nc.compile()
    return nc


_NC_CACHE = None


def _get_program():
    global _NC_CACHE
    if _NC_CACHE is None:
        _NC_CACHE = build_program()
    return _NC_CACHE


def _prep(image_features, domain_labels, W1, W2, text_features, logit_scale):
    """Host-side: sort rows by domain, gather per-block weights, compute
    exact logits for the few rows that land in a mixed block with the
    wrong expert."""
    x = np.asarray(image_features, dtype=np.float32)
    labels = np.asarray(domain_labels).astype(np.int64)
    W1 = np.asarray(W1, dtype=np.float32)
    W2 = np.asarray(W2, dtype=np.float32)
    T = np.asarray(text_features, dtype=np.float32)
    ls = float(np.asarray(logit_scale))

    perm = np.argsort(labels, kind="stable")
    lab_s = labels[perm]
    blk_dom = np.empty(NBLK, dtype=np.int64)
    fix_sorted = np.zeros(B, dtype=bool)
    for i in range(NBLK):
        seg = lab_s[i * RB : (i + 1) * RB]
        counts = np.bincount(seg, minlength=ND)
        m = int(counts.argmax())
        blk_dom[i] = m
        if counts[m] != RB:
            fix_sorted[i * RB : (i + 1) * RB] = seg != m

    x_s = x[perm]                                           # [B, D]
    xt_full = np.ascontiguousarray(x_s.T).astype(BF16NP)    # [D, B]
    w1_b = W1.astype(BF16NP)                                # [ND, D, R]
    w2_b = np.ascontiguousarray(0.25 * W2).astype(BF16NP)   # [ND, R, D]
    ttp = np.zeros((D, NTP), dtype=np.float32)
    ttp[:, :NT] = T.T
    tt_b = ttp.astype(BF16NP)
    sc = np.array([[np.exp(-2.0 * ls)]], dtype=np.float32)
    oc = np.ones((128, 1), dtype=np.float32)
    orow = np.ones((1, 128), dtype=np.float32)

    in_maps = []
    for c in range(NC):
        cols = slice(c * BPC, (c + 1) * BPC)
        doms = blk_dom[c * NB : (c + 1) * NB]
        in_maps.append(
            {
                "xt": np.ascontiguousarray(xt_full[:, cols]),
                "w1b": np.ascontiguousarray(w1_b[doms]),
                "w2b": np.ascontiguousarray(w2_b[doms]),
                "tt": tt_b,
                "sc": sc,
                "oc": oc,
                "orow": orow,
            }
        )

    # exact recompute for minority rows of mixed blocks
    fix_orig = perm[fix_sorted]
    fixed = np.empty((fix_orig.size, NT), dtype=np.float32)
    if fix_orig.size:
        xe = x[fix_orig]
        le = labels[fix_orig]
        for dcur in range(ND):
            m = le == dcur
            if not m.any():
                continue
            xm = xe[m]
            h = np.maximum(xm @ W1[dcur], 0.0)
            a = np.maximum(h @ W2[dcur], 0.0)
            f = 0.2 * a + 0.8 * xm
            f /= np.linalg.norm(f, axis=1, keepdims=True)
            fixed[m] = np.exp(ls) * (f @ T.T)
    return in_maps, perm, fix_orig, fixed


def make_in_maps(image_features, domain_labels, W1, W2, text_features, logit_scale):
    in_maps, _, _, _ = _prep(
        image_features, domain_labels, W1, W2, text_features, logit_scale
    )
    return in_maps


def kernel(image_features, domain_labels, W1, W2, text_features, logit_scale, **kw):
    in_maps, perm, fix_orig, fixed = _prep(
        image_features, domain_labels, W1, W2, text_features, logit_scale
    )
    nc = _get_program()
    res = run_bass_kernel_spmd(nc, in_maps, list(range(NC)))

    out_sorted = np.empty((B, NT), dtype=np.float32)
    for c in range(NC):
        out_sorted[c * BPC : (c + 1) * BPC, :] = res.results[c]["ot"][:NT, :].T
    out = np.empty((B, NT), dtype=np.float32)
    out[perm] = out_sorted
    if fix_orig.size:
        out[fix_orig] = fixed
    return out
